# revision 46
# baseline (speedup 1.0000x reference)
"""Trainium2 kernel for nn_DirectPerm: permutation-based cis-scan max-TS.

Math: for each permutation p, the reference computes
    TS_p = max_l |beta_lp| / se_lp
with beta = (Gt.T yt)/gtg, se = sqrt(rss/dof/gtg), rss = yty - beta^2 gtg.
Substituting s = beta^2 gtg = C^2 with C = Gn.T yt and Gn = Gt / sqrt(gtg):
    TS_lp = sqrt(dof * s / (yty_p - s)),  monotone increasing in s,
so TS_p = sqrt(dof * s*_p / (yty_p - s*_p)) with s*_p = max_l C_lp^2.

Device work: C = Yt.T @ Gn (perms on PSUM partitions, variants on the free
dim), then one abs-max tensor_reduce per (chunk, perm-tile) group. The
variant dimension L is sharded 8 ways across the NeuronCores; the host takes
an elementwise max over the 8 per-core [P] vectors and finishes the tiny
scalar tail exactly as the fp32 jax reference does.

Layout: the l dimension per core (1250 -> 1280) is split into chunks of
512/512/256 so each accumulation group is a single PSUM bank; all 8 perm
tiles of a chunk are in flight at once (8 banks), letting the PE consume
sample-tiles (k) in DMA arrival order. Inputs stream as A_k = [Yt_k | Gn_k
chunk0] then B_k = Gn_k chunk1, C_k = Gn_k chunk2, so matmuls start ~2us
after the first DMA lands and the PE never starves afterwards.
"""

import numpy as np

# Problem geometry (hardcoded per the harness contract).
N_SAMP = 1000
N_COV = 10
L_FULL = 10000
N_CORES = 8

PAD_SAMP = 1024   # samples padded to 8 k-tiles of 128
PAD_PERM = 1024   # permutations padded to 8 m-tiles of 128
L_SHARD = L_FULL // N_CORES      # 1250 variants per core
KTILES = PAD_SAMP // 128
MTILES = PAD_PERM // 128
CHUNKS = [512, 512, 256]   # last chunk zero-padded 226 -> 256

MM_DTYPE = "float16"

_BUILT = {}


def _legalize_waits(nc):
    """Walrus codegen allows only ONE sync-wait on compute instructions
    (e.g. the Matmult weight-load slot, TensorReduce) and two on
    EventSemaphore. Hoist larger wait sets into standalone EventSemaphore
    instructions on the same engine immediately before the instruction —
    identical semantics, no codegen limit."""
    from concourse import mybir

    n_fixed = 0
    for fn in nc.m.functions:
        for b in fn.blocks:
            il = list(b.instructions)
            out = []
            changed = False
            for inst in il:
                si = getattr(inst, "sync_info", None)
                waits = list(si.on_wait) if si is not None and si.on_wait else []
                if (not isinstance(inst, mybir.InstEventSemaphore)
                        and len(waits) > 1):
                    for j in range(0, len(waits), 2):
                        out.append(mybir.InstEventSemaphore(
                            name=f"{inst.name}-waitsplit{j}",
                            opcode="EventSemaphore",
                            engine=inst.engine,
                            sync_info=mybir.SyncInfo(
                                on_wait=waits[j:j + 2], on_update=[]),
                        ))
                    inst.sync_info = mybir.SyncInfo(
                        on_wait=[], on_update=list(si.on_update))
                    n_fixed += 1
                    changed = True
                elif isinstance(inst, mybir.InstEventSemaphore) and len(waits) > 2:
                    for j in range(2, len(waits), 2):
                        out.append(mybir.InstEventSemaphore(
                            name=f"{inst.name}-waitsplit{j}",
                            opcode="EventSemaphore",
                            engine=inst.engine,
                            sync_info=mybir.SyncInfo(
                                on_wait=waits[j:j + 2], on_update=[]),
                        ))
                    inst.sync_info = mybir.SyncInfo(
                        on_wait=waits[:2], on_update=list(si.on_update))
                    n_fixed += 1
                    changed = True
                out.append(inst)
            if changed:
                b.instructions = out
    return n_fixed


def _build_nc(mm_dtype_name=MM_DTYPE):
    """Build the SPMD Bass program (same on every core)."""
    from contextlib import ExitStack
    import concourse.bass as bass
    import concourse.tile as tile
    from concourse import mybir

    mm_dt = getattr(mybir.dt, mm_dtype_name)
    f32 = mybir.dt.float32
    AW = PAD_PERM + CHUNKS[0]   # A pack: chunk-0 gn columns then yt columns

    BW = CHUNKS[1] + CHUNKS[2]  # B pack: chunk-1 then chunk-2 gn columns

    nc = bass.Bass()
    a_ext = nc.declare_dram_parameter("pa", [KTILES, 128, AW], mm_dt, isOutput=False)
    b_ext = nc.declare_dram_parameter("pb", [KTILES, 128, BW], mm_dt, isOutput=False)
    out_ext = nc.declare_dram_parameter("smax", [128, MTILES], f32, isOutput=True)

    with ExitStack() as ctx:
        tc = ctx.enter_context(tile.TileContext(nc))
        a_pool = ctx.enter_context(tc.tile_pool(name="pa", bufs=1))
        b_pool = ctx.enter_context(tc.tile_pool(name="pb", bufs=1))
        ps_pool = ctx.enter_context(tc.tile_pool(name="ps", bufs=8, space="PSUM"))
        red_pool = ctx.enter_context(tc.tile_pool(name="red", bufs=4))
        out_pool = ctx.enter_context(tc.tile_pool(name="out", bufs=1))

        # A pack layout: [gn_c0 (512) | yt (1024)] so the first half-tile DMA
        # already carries chunk-0 gn plus the first perm tiles.
        a_sb, b_sb = [], []
        for k in range(KTILES):
            t = a_pool.tile([128, AW], mm_dt, tag=f"pa{k}")
            if k == 0:
                nc.sync.dma_start(t[:, 0:768], a_ext[k][:, 0:768])
                nc.sync.dma_start(t[:, 768:AW], a_ext[k][:, 768:AW])
            else:
                nc.sync.dma_start(t[:], a_ext[k])
            a_sb.append(t)
        # Gate the B-pack DMAs to ~12us so their packets don't steal HWDGE
        # bandwidth from the A tiles the PE consumes first (B is needed from
        # ~24us; its 1.2MB lands in ~3us).
        with tc.tile_wait_until(0.012):
            for k in range(KTILES):
                t = b_pool.tile([128, BW], mm_dt, tag=f"pb{k}")
                nc.sync.dma_start(t[:], b_ext[k])
                b_sb.append(t)

        def lhsT(k, m):
            return a_sb[k][:, CHUNKS[0] + m * 128:CHUNKS[0] + (m + 1) * 128]

        def rhs(c, k, w):
            if c == 0:
                return a_sb[k][:, 0:w]
            if c == 1:
                return b_sb[k][:, 0:w]
            return b_sb[k][:, CHUNKS[1]:CHUNKS[1] + w]

        # partial |C| maxes per (chunk, perm-tile): column c*MTILES+m
        pm = out_pool.tile([128, 3 * MTILES], f32, tag="pm")
        smax_sb = out_pool.tile([128, MTILES], f32, tag="smax")

        # Short HAM pre-warm: a few junk matmuls fill the gap between the PE
        # preamble (~7.5us) and the first A-tile arrival (~10us), banking
        # clock-gate activity credit so fewer REAL matmuls run at the cold
        # half-clock. (A full 9-MM warm-up delays real work; none leaves ~7
        # real matmuls cold — 4 measured best.)
        warm = out_pool.tile([128, 512], mm_dt, tag="warm")
        nc.vector.memset(warm[:], 0.0)
        wps = ps_pool.tile([128, 512], f32, tag="ps")
        for i in range(4):
            nc.tensor.matmul(wps[:], warm[:, 0:128], warm[:],
                             start=(i == 0), stop=(i == 3))

        for c, w in enumerate(CHUNKS):
            for m in range(MTILES):
                ps = ps_pool.tile([128, 512], f32, tag="ps")
                for k in range(KTILES):
                    nc.tensor.matmul(
                        ps[:, 0:w], lhsT(k, m), rhs(c, k, w),
                        start=(k == 0), stop=(k == KTILES - 1),
                    )
                nc.vector.tensor_reduce(
                    pm[:, c * MTILES + m:c * MTILES + m + 1], ps[:, 0:w],
                    axis=mybir.AxisListType.X, op=mybir.AluOpType.max,
                    apply_absolute_value=True,
                )
                if c == 2:
                    # all three partials for this perm tile are in; combine.
                    # smax holds max|C| — the host squares it.
                    nc.vector.tensor_reduce(
                        smax_sb[:, m:m + 1], pm[:, m::MTILES],
                        axis=mybir.AxisListType.X, op=mybir.AluOpType.max,
                    )
        nc.sync.dma_start(out_ext[:], smax_sb[:])
    _legalize_waits(nc)
    _strip_final_barrier(nc)
    return nc


def _strip_final_barrier(nc):
    """Drop the second all-engine barrier in Tile's end block (everything
    after the semaphore range-clear ISA op). It only delays per-engine
    stream ends until the clear completes; the runtime's own epilogue
    re-zeroes every semaphore anyway."""
    from concourse import mybir

    for fn in nc.m.functions:
        for b in fn.blocks:
            if not b.name.endswith("_end"):
                continue
            il = list(b.instructions)
            isa_idx = [i for i, inst in enumerate(il)
                       if isinstance(inst, mybir.InstISA)]
            if isa_idx:
                b.instructions = il[:isa_idx[-1] + 1]


def _get_nc(mm_dtype_name=MM_DTYPE):
    if mm_dtype_name not in _BUILT:
        _BUILT[mm_dtype_name] = _build_nc(mm_dtype_name)
    return _BUILT[mm_dtype_name]


def _round_tf32(a):
    """Round-to-nearest-even fp32 -> fp32r (11 explicit mantissa bits)."""
    u = np.ascontiguousarray(a, dtype=np.float32).view(np.uint32)
    r = (u + np.uint32(0x7FF) + ((u >> np.uint32(12)) & np.uint32(1))) & np.uint32(0xFFFFF000)
    return r.view(np.float32)


def _permuted_y(y, key_seed, n_perm):
    """Exactly replicate the reference's permutation stream (jax threefry)."""
    import jax
    import jax.numpy as jnp

    cpu = jax.devices("cpu")[0]
    with jax.default_device(cpu):
        y_j = jnp.asarray(np.asarray(y, dtype=np.float32))
        key0 = jax.random.PRNGKey(int(key_seed))

        def step(key, _):
            key, p_key = jax.random.split(key)
            return key, jax.random.permutation(p_key, y_j, axis=0)

        _, yp = jax.lax.scan(step, key0, xs=None, length=int(n_perm))
        return np.asarray(yp)  # [P, N]


def prepare_inputs(X, y, G, key_seed, P, mm_dtype_name=MM_DTYPE):
    """Host prep: residualize, normalize, permute, pad, shard.

    Returns (in_maps, yty[P] float64).
    """
    X = np.asarray(X, dtype=np.float32)
    y = np.asarray(y, dtype=np.float32)
    G = np.asarray(G, dtype=np.float32)
    n = X.shape[0]
    assert n == N_SAMP and G.shape[1] == L_FULL and P <= PAD_PERM

    X64 = X.astype(np.float64)
    XtX = X64.T @ X64
    XtX_inv = np.linalg.inv(XtX)

    def resid(V):
        return V - X64 @ (XtX_inv @ (X64.T @ V))

    Gt = resid(G.astype(np.float64))                 # [N, L]
    gtg = np.sum(Gt * Gt, axis=0)                    # [L]
    Gn = (Gt / np.sqrt(gtg)).astype(np.float32)      # [N, L] unit-norm columns

    Yp = _permuted_y(y, key_seed, P).T               # [N, P]
    Yt64 = resid(Yp.astype(np.float64))
    yty = np.sum(Yt64 * Yt64, axis=0)                # [P] float64
    Yt = Yt64.astype(np.float32)

    if mm_dtype_name == "float16":
        npdt = np.float16
        cast = lambda a: a.astype(np.float16)
    else:
        npdt = np.float32
        cast = _round_tf32

    AW = PAD_PERM + CHUNKS[0]
    BW = CHUNKS[1] + CHUNKS[2]
    Ytq = cast(Yt)                                   # [n, P]
    Gq = cast(Gn).reshape(n, N_CORES, L_SHARD).transpose(1, 0, 2)  # [cores, n, 1250]

    pa = np.zeros((N_CORES, PAD_SAMP, AW), dtype=npdt)
    pa[:, :n, 0:CHUNKS[0]] = Gq[:, :, 0:512]
    pa[:, :n, CHUNKS[0]:CHUNKS[0] + P] = Ytq[None]
    pb = np.zeros((N_CORES, PAD_SAMP, BW), dtype=npdt)
    pb[:, :n, 0:L_SHARD - 512] = Gq[:, :, 512:L_SHARD]
    in_maps = [
        {
            "pa": pa[c].reshape(KTILES, 128, AW),
            "pb": pb[c].reshape(KTILES, 128, BW),
        }
        for c in range(N_CORES)
    ]
    return in_maps, yty


def postprocess(results, yty, obs_p, P, n=N_SAMP, k_cov=N_COV):
    """Gather device outputs and finish the tiny fp32 tail like the reference."""
    import jax
    import jax.numpy as jnp
    import jax.scipy.stats as jaxstats

    smax_cores = np.stack([
        np.asarray(r["smax"]).T.reshape(-1)[:P] for r in results
    ])                                               # [cores, P] max|C|
    s = np.max(smax_cores, axis=0).astype(np.float64) ** 2

    dof = float(n - k_cov - 1)
    TS = np.sqrt(dof * s / (yty - s)).astype(np.float32)
    cpu = jax.devices("cpu")[0]
    with jax.default_device(cpu):
        pvals = np.asarray(
            1.0 - jaxstats.chi2.cdf(jnp.square(jnp.asarray(TS)), 1.0),
            dtype=np.float32,
        )
    obs = np.asarray(obs_p, dtype=np.float32)
    adj_p = np.float32((np.sum(pvals < obs) + 1) / (P + 1))
    return adj_p, pvals, TS


def kernel(X, y, G, obs_p, key_seed, max_perm_direct):
    from concourse.bass_utils import run_bass_kernel_spmd

    P = int(np.asarray(max_perm_direct))
    seed = int(np.asarray(key_seed))
    in_maps, yty = prepare_inputs(X, y, G, seed, P)
    nc = _get_nc()
    res = run_bass_kernel_spmd(nc, in_maps, list(range(N_CORES)))
    return postprocess(res.results, yty, obs_p, P)


# revision 48
# speedup vs baseline: 1.0366x; 1.0366x over previous
"""Trainium2 kernel for nn_DirectPerm: permutation-based cis-scan max-TS.

Math: for each permutation p, the reference computes
    TS_p = max_l |beta_lp| / se_lp
with beta = (Gt.T yt)/gtg, se = sqrt(rss/dof/gtg), rss = yty - beta^2 gtg.
Substituting s = beta^2 gtg = C^2 with C = Gn.T yt and Gn = Gt / sqrt(gtg):
    TS_lp = sqrt(dof * s / (yty_p - s)),  monotone increasing in s,
so TS_p = sqrt(dof * s*_p / (yty_p - s*_p)) with s*_p = max_l C_lp^2.

Device work: C = Yt.T @ Gn (perms on PSUM partitions, variants on the free
dim), then one abs-max tensor_reduce per (chunk, perm-tile) group. The
variant dimension L is sharded 8 ways across the NeuronCores; the host takes
an elementwise max over the 8 per-core [P] vectors and finishes the tiny
scalar tail exactly as the fp32 jax reference does.

Layout: the l dimension per core (1250 -> 1280) is split into chunks of
512/512/256 so each accumulation group is a single PSUM bank; all 8 perm
tiles of a chunk are in flight at once (8 banks), letting the PE consume
sample-tiles (k) in DMA arrival order. Inputs stream as A_k = [Yt_k | Gn_k
chunk0] then B_k = Gn_k chunk1, C_k = Gn_k chunk2, so matmuls start ~2us
after the first DMA lands and the PE never starves afterwards.
"""

import numpy as np

# Problem geometry (hardcoded per the harness contract).
N_SAMP = 1000
N_COV = 10
L_FULL = 10000
N_CORES = 8

PAD_SAMP = 1024   # samples padded to 8 k-tiles of 128
PAD_PERM = 1024   # permutations padded to 8 m-tiles of 128
L_SHARD = L_FULL // N_CORES      # 1250 variants per core
KTILES = PAD_SAMP // 128
MTILES = PAD_PERM // 128
CHUNKS = [512, 512, 256]   # last chunk zero-padded 226 -> 256

MM_DTYPE = "float16"

_BUILT = {}


def _legalize_waits(nc):
    """Walrus codegen allows only ONE sync-wait on compute instructions
    (e.g. the Matmult weight-load slot, TensorReduce) and two on
    EventSemaphore. Hoist larger wait sets into standalone EventSemaphore
    instructions on the same engine immediately before the instruction —
    identical semantics, no codegen limit."""
    from concourse import mybir

    n_fixed = 0
    for fn in nc.m.functions:
        for b in fn.blocks:
            il = list(b.instructions)
            out = []
            changed = False
            for inst in il:
                si = getattr(inst, "sync_info", None)
                waits = list(si.on_wait) if si is not None and si.on_wait else []
                if (not isinstance(inst, mybir.InstEventSemaphore)
                        and len(waits) > 1):
                    for j in range(0, len(waits), 2):
                        out.append(mybir.InstEventSemaphore(
                            name=f"{inst.name}-waitsplit{j}",
                            opcode="EventSemaphore",
                            engine=inst.engine,
                            sync_info=mybir.SyncInfo(
                                on_wait=waits[j:j + 2], on_update=[]),
                        ))
                    inst.sync_info = mybir.SyncInfo(
                        on_wait=[], on_update=list(si.on_update))
                    n_fixed += 1
                    changed = True
                elif isinstance(inst, mybir.InstEventSemaphore) and len(waits) > 2:
                    for j in range(2, len(waits), 2):
                        out.append(mybir.InstEventSemaphore(
                            name=f"{inst.name}-waitsplit{j}",
                            opcode="EventSemaphore",
                            engine=inst.engine,
                            sync_info=mybir.SyncInfo(
                                on_wait=waits[j:j + 2], on_update=[]),
                        ))
                    inst.sync_info = mybir.SyncInfo(
                        on_wait=waits[:2], on_update=list(si.on_update))
                    n_fixed += 1
                    changed = True
                out.append(inst)
            if changed:
                b.instructions = out
    return n_fixed


def _build_nc(mm_dtype_name=MM_DTYPE):
    """Build the SPMD Bass program (same on every core)."""
    from contextlib import ExitStack
    import concourse.bass as bass
    import concourse.tile as tile
    from concourse import mybir

    mm_dt = getattr(mybir.dt, mm_dtype_name)
    f32 = mybir.dt.float32
    AW = PAD_PERM + CHUNKS[0]   # A pack: chunk-0 gn columns then yt columns

    BW = CHUNKS[1] + CHUNKS[2]  # B pack: chunk-1 then chunk-2 gn columns

    nc = bass.Bass()
    a_ext = nc.declare_dram_parameter("pa", [KTILES, 128, AW], mm_dt, isOutput=False)
    b_ext = nc.declare_dram_parameter("pb", [KTILES, 128, BW], mm_dt, isOutput=False)
    out_ext = nc.declare_dram_parameter("smax", [128, MTILES], f32, isOutput=True)

    with ExitStack() as ctx:
        tc = ctx.enter_context(tile.TileContext(nc))
        a_pool = ctx.enter_context(tc.tile_pool(name="pa", bufs=1))
        b_pool = ctx.enter_context(tc.tile_pool(name="pb", bufs=1))
        ps_pool = ctx.enter_context(tc.tile_pool(name="ps", bufs=8, space="PSUM"))
        red_pool = ctx.enter_context(tc.tile_pool(name="red", bufs=4))
        out_pool = ctx.enter_context(tc.tile_pool(name="out", bufs=1))

        # A pack layout: [gn_c0 (512) | yt (1024)] so the first half-tile DMA
        # already carries chunk-0 gn plus the first perm tiles.
        a_sb, b_sb = [], []
        for k in range(KTILES):
            t = a_pool.tile([128, AW], mm_dt, tag=f"pa{k}")
            if k == 0:
                nc.sync.dma_start(t[:, 0:768], a_ext[k][:, 0:768])
                nc.sync.dma_start(t[:, 768:AW], a_ext[k][:, 768:AW])
            else:
                nc.sync.dma_start(t[:], a_ext[k])
            a_sb.append(t)
        # Gate the B-pack DMAs to ~12us so their packets don't steal HWDGE
        # bandwidth from the A tiles the PE consumes first (B is needed from
        # ~24us; its 1.2MB lands in ~3us).
        with tc.tile_wait_until(0.012):
            for k in range(KTILES):
                t = b_pool.tile([128, BW], mm_dt, tag=f"pb{k}")
                nc.sync.dma_start(t[:], b_ext[k])
                b_sb.append(t)

        def lhsT(k, m):
            return a_sb[k][:, CHUNKS[0] + m * 128:CHUNKS[0] + (m + 1) * 128]

        def rhs(c, k, w):
            if c == 0:
                return a_sb[k][:, 0:w]
            if c == 1:
                return b_sb[k][:, 0:w]
            return b_sb[k][:, CHUNKS[1]:CHUNKS[1] + w]

        # partial |C| maxes per (chunk, perm-tile): column c*MTILES+m
        pm = out_pool.tile([128, 3 * MTILES], f32, tag="pm")
        smax_sb = out_pool.tile([128, MTILES], f32, tag="smax")

        # Short HAM pre-warm: a few junk matmuls fill the gap between the PE
        # preamble (~7.5us) and the first A-tile arrival (~10us), banking
        # clock-gate activity credit so fewer REAL matmuls run at the cold
        # half-clock. (A full 9-MM warm-up delays real work; none leaves ~7
        # real matmuls cold — 3 measured best across the 0/3/4/6/9 sweep.)
        warm = out_pool.tile([128, 512], mm_dt, tag="warm")
        nc.vector.memset(warm[:], 0.0)
        wps = ps_pool.tile([128, 512], f32, tag="ps")
        for i in range(3):
            nc.tensor.matmul(wps[:], warm[:, 0:128], warm[:],
                             start=(i == 0), stop=(i == 2))

        for c, w in enumerate(CHUNKS):
            for m in range(MTILES):
                ps = ps_pool.tile([128, 512], f32, tag="ps")
                for k in range(KTILES):
                    nc.tensor.matmul(
                        ps[:, 0:w], lhsT(k, m), rhs(c, k, w),
                        start=(k == 0), stop=(k == KTILES - 1),
                    )
                nc.vector.tensor_reduce(
                    pm[:, c * MTILES + m:c * MTILES + m + 1], ps[:, 0:w],
                    axis=mybir.AxisListType.X, op=mybir.AluOpType.max,
                    apply_absolute_value=True,
                )
                if c == 2:
                    # all three partials for this perm tile are in; combine.
                    # smax holds max|C| — the host squares it.
                    nc.vector.tensor_reduce(
                        smax_sb[:, m:m + 1], pm[:, m::MTILES],
                        axis=mybir.AxisListType.X, op=mybir.AluOpType.max,
                    )
        nc.sync.dma_start(out_ext[:], smax_sb[:])
    _legalize_waits(nc)
    _strip_final_barrier(nc)
    return nc


def _strip_final_barrier(nc):
    """Drop the second all-engine barrier in Tile's end block (everything
    after the semaphore range-clear ISA op). It only delays per-engine
    stream ends until the clear completes; the runtime's own epilogue
    re-zeroes every semaphore anyway."""
    from concourse import mybir

    for fn in nc.m.functions:
        for b in fn.blocks:
            if not b.name.endswith("_end"):
                continue
            il = list(b.instructions)
            isa_idx = [i for i, inst in enumerate(il)
                       if isinstance(inst, mybir.InstISA)]
            if isa_idx:
                b.instructions = il[:isa_idx[-1] + 1]


def _get_nc(mm_dtype_name=MM_DTYPE):
    if mm_dtype_name not in _BUILT:
        _BUILT[mm_dtype_name] = _build_nc(mm_dtype_name)
    return _BUILT[mm_dtype_name]


def _round_tf32(a):
    """Round-to-nearest-even fp32 -> fp32r (11 explicit mantissa bits)."""
    u = np.ascontiguousarray(a, dtype=np.float32).view(np.uint32)
    r = (u + np.uint32(0x7FF) + ((u >> np.uint32(12)) & np.uint32(1))) & np.uint32(0xFFFFF000)
    return r.view(np.float32)


def _permuted_y(y, key_seed, n_perm):
    """Exactly replicate the reference's permutation stream (jax threefry)."""
    import jax
    import jax.numpy as jnp

    cpu = jax.devices("cpu")[0]
    with jax.default_device(cpu):
        y_j = jnp.asarray(np.asarray(y, dtype=np.float32))
        key0 = jax.random.PRNGKey(int(key_seed))

        def step(key, _):
            key, p_key = jax.random.split(key)
            return key, jax.random.permutation(p_key, y_j, axis=0)

        _, yp = jax.lax.scan(step, key0, xs=None, length=int(n_perm))
        return np.asarray(yp)  # [P, N]


def prepare_inputs(X, y, G, key_seed, P, mm_dtype_name=MM_DTYPE):
    """Host prep: residualize, normalize, permute, pad, shard.

    Returns (in_maps, yty[P] float64).
    """
    X = np.asarray(X, dtype=np.float32)
    y = np.asarray(y, dtype=np.float32)
    G = np.asarray(G, dtype=np.float32)
    n = X.shape[0]
    assert n == N_SAMP and G.shape[1] == L_FULL and P <= PAD_PERM

    X64 = X.astype(np.float64)
    XtX = X64.T @ X64
    XtX_inv = np.linalg.inv(XtX)

    def resid(V):
        return V - X64 @ (XtX_inv @ (X64.T @ V))

    Gt = resid(G.astype(np.float64))                 # [N, L]
    gtg = np.sum(Gt * Gt, axis=0)                    # [L]
    Gn = (Gt / np.sqrt(gtg)).astype(np.float32)      # [N, L] unit-norm columns

    Yp = _permuted_y(y, key_seed, P).T               # [N, P]
    Yt64 = resid(Yp.astype(np.float64))
    yty = np.sum(Yt64 * Yt64, axis=0)                # [P] float64
    Yt = Yt64.astype(np.float32)

    if mm_dtype_name == "float16":
        npdt = np.float16
        cast = lambda a: a.astype(np.float16)
    else:
        npdt = np.float32
        cast = _round_tf32

    AW = PAD_PERM + CHUNKS[0]
    BW = CHUNKS[1] + CHUNKS[2]
    Ytq = cast(Yt)                                   # [n, P]
    Gq = cast(Gn).reshape(n, N_CORES, L_SHARD).transpose(1, 0, 2)  # [cores, n, 1250]

    pa = np.zeros((N_CORES, PAD_SAMP, AW), dtype=npdt)
    pa[:, :n, 0:CHUNKS[0]] = Gq[:, :, 0:512]
    pa[:, :n, CHUNKS[0]:CHUNKS[0] + P] = Ytq[None]
    pb = np.zeros((N_CORES, PAD_SAMP, BW), dtype=npdt)
    pb[:, :n, 0:L_SHARD - 512] = Gq[:, :, 512:L_SHARD]
    in_maps = [
        {
            "pa": pa[c].reshape(KTILES, 128, AW),
            "pb": pb[c].reshape(KTILES, 128, BW),
        }
        for c in range(N_CORES)
    ]
    return in_maps, yty


def postprocess(results, yty, obs_p, P, n=N_SAMP, k_cov=N_COV):
    """Gather device outputs and finish the tiny fp32 tail like the reference."""
    import jax
    import jax.numpy as jnp
    import jax.scipy.stats as jaxstats

    smax_cores = np.stack([
        np.asarray(r["smax"]).T.reshape(-1)[:P] for r in results
    ])                                               # [cores, P] max|C|
    s = np.max(smax_cores, axis=0).astype(np.float64) ** 2

    dof = float(n - k_cov - 1)
    TS = np.sqrt(dof * s / (yty - s)).astype(np.float32)
    cpu = jax.devices("cpu")[0]
    with jax.default_device(cpu):
        pvals = np.asarray(
            1.0 - jaxstats.chi2.cdf(jnp.square(jnp.asarray(TS)), 1.0),
            dtype=np.float32,
        )
    obs = np.asarray(obs_p, dtype=np.float32)
    adj_p = np.float32((np.sum(pvals < obs) + 1) / (P + 1))
    return adj_p, pvals, TS


def kernel(X, y, G, obs_p, key_seed, max_perm_direct):
    from concourse.bass_utils import run_bass_kernel_spmd

    P = int(np.asarray(max_perm_direct))
    seed = int(np.asarray(key_seed))
    in_maps, yty = prepare_inputs(X, y, G, seed, P)
    nc = _get_nc()
    res = run_bass_kernel_spmd(nc, in_maps, list(range(N_CORES)))
    return postprocess(res.results, yty, obs_p, P)


# revision 54
# speedup vs baseline: 1.0543x; 1.0171x over previous
"""Trainium2 kernel for nn_DirectPerm: permutation-based cis-scan max-TS.

Math: for each permutation p, the reference computes
    TS_p = max_l |beta_lp| / se_lp
with beta = (Gt.T yt)/gtg, se = sqrt(rss/dof/gtg), rss = yty - beta^2 gtg.
Substituting s = beta^2 gtg = C^2 with C = Gn.T yt and Gn = Gt / sqrt(gtg):
    TS_lp = sqrt(dof * s / (yty_p - s)),  monotone increasing in s,
so TS_p = sqrt(dof * s*_p / (yty_p - s*_p)) with s*_p = max_l C_lp^2.

Device work: C = Yt.T @ Gn (perms on PSUM partitions, variants on the free
dim), then one abs-max tensor_reduce per (chunk, perm-tile) group. The
variant dimension L is sharded 8 ways across the NeuronCores; the host takes
an elementwise max over the 8 per-core [P] vectors and finishes the tiny
scalar tail exactly as the fp32 jax reference does.

Layout: the l dimension per core (1250 -> 1280) is split into chunks of
512/512/256 so each accumulation group is a single PSUM bank; all 8 perm
tiles of a chunk are in flight at once (8 banks), letting the PE consume
sample-tiles (k) in DMA arrival order. Inputs stream as A_k = [Yt_k | Gn_k
chunk0] then B_k = Gn_k chunk1, C_k = Gn_k chunk2, so matmuls start ~2us
after the first DMA lands and the PE never starves afterwards.
"""

import numpy as np

# Problem geometry (hardcoded per the harness contract).
N_SAMP = 1000
N_COV = 10
L_FULL = 10000
N_CORES = 8

PAD_SAMP = 1024   # samples padded to 8 k-tiles of 128
PAD_PERM = 1024   # permutations padded to 8 m-tiles of 128
L_SHARD = L_FULL // N_CORES      # 1250 variants per core
KTILES = PAD_SAMP // 128
MTILES = PAD_PERM // 128
CHUNKS = [512, 512, 256]   # last chunk zero-padded 226 -> 256

MM_DTYPE = "float16"

_BUILT = {}


def _legalize_waits(nc):
    """Walrus codegen allows only ONE sync-wait on compute instructions
    (e.g. the Matmult weight-load slot, TensorReduce) and two on
    EventSemaphore. Hoist larger wait sets into standalone EventSemaphore
    instructions on the same engine immediately before the instruction —
    identical semantics, no codegen limit."""
    from concourse import mybir

    n_fixed = 0
    for fn in nc.m.functions:
        for b in fn.blocks:
            il = list(b.instructions)
            out = []
            changed = False
            for inst in il:
                si = getattr(inst, "sync_info", None)
                waits = list(si.on_wait) if si is not None and si.on_wait else []
                if (not isinstance(inst, mybir.InstEventSemaphore)
                        and len(waits) > 1):
                    for j in range(0, len(waits), 2):
                        out.append(mybir.InstEventSemaphore(
                            name=f"{inst.name}-waitsplit{j}",
                            opcode="EventSemaphore",
                            engine=inst.engine,
                            sync_info=mybir.SyncInfo(
                                on_wait=waits[j:j + 2], on_update=[]),
                        ))
                    inst.sync_info = mybir.SyncInfo(
                        on_wait=[], on_update=list(si.on_update))
                    n_fixed += 1
                    changed = True
                elif isinstance(inst, mybir.InstEventSemaphore) and len(waits) > 2:
                    for j in range(2, len(waits), 2):
                        out.append(mybir.InstEventSemaphore(
                            name=f"{inst.name}-waitsplit{j}",
                            opcode="EventSemaphore",
                            engine=inst.engine,
                            sync_info=mybir.SyncInfo(
                                on_wait=waits[j:j + 2], on_update=[]),
                        ))
                    inst.sync_info = mybir.SyncInfo(
                        on_wait=waits[:2], on_update=list(si.on_update))
                    n_fixed += 1
                    changed = True
                out.append(inst)
            if changed:
                b.instructions = out
    return n_fixed


def _build_nc(mm_dtype_name=MM_DTYPE):
    """Build the SPMD Bass program (same on every core)."""
    from contextlib import ExitStack
    import concourse.bass as bass
    import concourse.tile as tile
    from concourse import mybir

    mm_dt = getattr(mybir.dt, mm_dtype_name)
    f32 = mybir.dt.float32
    AW = PAD_PERM + CHUNKS[0]   # A pack: chunk-0 gn columns then yt columns

    BW = CHUNKS[1] + CHUNKS[2]  # B pack: chunk-1 then chunk-2 gn columns

    nc = bass.Bass()
    a_ext = nc.declare_dram_parameter("pa", [KTILES, 128, AW], mm_dt, isOutput=False)
    b_ext = nc.declare_dram_parameter("pb", [KTILES, 128, BW], mm_dt, isOutput=False)
    out_ext = nc.declare_dram_parameter("smax", [128, MTILES], f32, isOutput=True)

    with ExitStack() as ctx:
        tc = ctx.enter_context(tile.TileContext(nc))
        a_pool = ctx.enter_context(tc.tile_pool(name="pa", bufs=1))
        b_pool = ctx.enter_context(tc.tile_pool(name="pb", bufs=1))
        ps_pool = ctx.enter_context(tc.tile_pool(name="ps", bufs=8, space="PSUM"))
        red_pool = ctx.enter_context(tc.tile_pool(name="red", bufs=4))
        out_pool = ctx.enter_context(tc.tile_pool(name="out", bufs=1))

        # A pack layout: [gn_c0 (512) | yt (1024)] so the first half-tile DMA
        # already carries chunk-0 gn plus the first perm tiles.
        a_sb, b_sb = [], []
        for k in range(KTILES):
            t = a_pool.tile([128, AW], mm_dt, tag=f"pa{k}")
            if k == 0:
                nc.sync.dma_start(t[:, 0:768], a_ext[k][:, 0:768])
                nc.sync.dma_start(t[:, 768:AW], a_ext[k][:, 768:AW])
            else:
                nc.sync.dma_start(t[:], a_ext[k])
            a_sb.append(t)
        # Gate the B-pack DMAs to ~12us so their packets don't steal HWDGE
        # bandwidth from the A tiles the PE consumes first (B is needed from
        # ~24us; its 1.2MB lands in ~3us).
        with tc.tile_wait_until(0.012):
            for k in range(KTILES):
                t = b_pool.tile([128, BW], mm_dt, tag=f"pb{k}")
                nc.sync.dma_start(t[:], b_ext[k])
                b_sb.append(t)

        def lhsT(k, m):
            return a_sb[k][:, CHUNKS[0] + m * 128:CHUNKS[0] + (m + 1) * 128]

        def rhs(c, k, w):
            if c == 0:
                return a_sb[k][:, 0:w]
            if c == 1:
                return b_sb[k][:, 0:w]
            return b_sb[k][:, CHUNKS[1]:CHUNKS[1] + w]

        # partial |C| maxes per (chunk, perm-tile): column c*MTILES+m
        pm = out_pool.tile([128, 3 * MTILES], f32, tag="pm")
        smax_sb = out_pool.tile([128, MTILES], f32, tag="smax")

        # Short HAM pre-warm: a few junk matmuls fill the gap between the PE
        # preamble (~7.5us) and the first A-tile arrival (~10us), banking
        # clock-gate activity credit so fewer REAL matmuls run at the cold
        # half-clock. (A full 9-MM warm-up delays real work; none leaves ~7
        # real matmuls cold — 3 measured best across the 0/3/4/6/9 sweep.)
        warm = out_pool.tile([128, 512], mm_dt, tag="warm")
        nc.vector.memset(warm[:], 0.0)
        wps = ps_pool.tile([128, 512], f32, tag="ps")
        for i in range(3):
            nc.tensor.matmul(wps[:], warm[:, 0:128], warm[:],
                             start=(i == 0), stop=(i == 2))

        for c, w in enumerate(CHUNKS):
            for m in range(MTILES):
                ps = ps_pool.tile([128, 512], f32, tag="ps")
                for k in range(KTILES):
                    nc.tensor.matmul(
                        ps[:, 0:w], lhsT(k, m), rhs(c, k, w),
                        start=(k == 0), stop=(k == KTILES - 1),
                    )
                nc.vector.tensor_reduce(
                    pm[:, c * MTILES + m:c * MTILES + m + 1], ps[:, 0:w],
                    axis=mybir.AxisListType.X, op=mybir.AluOpType.max,
                    apply_absolute_value=True,
                )
                if c == 2:
                    # all three partials for this perm tile are in; combine.
                    # smax holds max|C| — the host squares it.
                    nc.vector.tensor_reduce(
                        smax_sb[:, m:m + 1], pm[:, m::MTILES],
                        axis=mybir.AxisListType.X, op=mybir.AluOpType.max,
                    )
        nc.sync.dma_start(out_ext[:], smax_sb[:])
    _legalize_waits(nc)
    _strip_final_barrier(nc)
    return nc


def _strip_final_barrier(nc):
    """Drop the second all-engine barrier in Tile's end block (everything
    after the semaphore range-clear ISA op). It only delays per-engine
    stream ends until the clear completes; the runtime's own epilogue
    re-zeroes every semaphore anyway."""
    from concourse import mybir

    for fn in nc.m.functions:
        for b in fn.blocks:
            if not b.name.endswith("_end"):
                continue
            il = list(b.instructions)
            isa_idx = [i for i, inst in enumerate(il)
                       if isinstance(inst, mybir.InstISA)]
            if isa_idx:
                b.instructions = il[:isa_idx[-1] + 1]


def _get_nc(mm_dtype_name=MM_DTYPE):
    if mm_dtype_name not in _BUILT:
        _BUILT[mm_dtype_name] = _build_nc(mm_dtype_name)
    return _BUILT[mm_dtype_name]


def _round_tf32(a):
    """Round-to-nearest-even fp32 -> fp32r (11 explicit mantissa bits)."""
    u = np.ascontiguousarray(a, dtype=np.float32).view(np.uint32)
    r = (u + np.uint32(0x7FF) + ((u >> np.uint32(12)) & np.uint32(1))) & np.uint32(0xFFFFF000)
    return r.view(np.float32)


def _permuted_y(y, key_seed, n_perm):
    """Exactly replicate the reference's permutation stream (jax threefry)."""
    import jax
    import jax.numpy as jnp

    cpu = jax.devices("cpu")[0]
    with jax.default_device(cpu):
        y_j = jnp.asarray(np.asarray(y, dtype=np.float32))
        key0 = jax.random.PRNGKey(int(key_seed))

        def step(key, _):
            key, p_key = jax.random.split(key)
            return key, jax.random.permutation(p_key, y_j, axis=0)

        _, yp = jax.lax.scan(step, key0, xs=None, length=int(n_perm))
        return np.asarray(yp)  # [P, N]


def prepare_inputs(X, y, G, key_seed, P, mm_dtype_name=MM_DTYPE):
    """Host prep: residualize, normalize, permute, pad, shard.

    Returns (in_maps, yty[P] float64).
    """
    X = np.asarray(X, dtype=np.float32)
    y = np.asarray(y, dtype=np.float32)
    G = np.asarray(G, dtype=np.float32)
    n = X.shape[0]
    assert n == N_SAMP and G.shape[1] == L_FULL and P <= PAD_PERM

    X64 = X.astype(np.float64)
    XtX = X64.T @ X64
    XtX_inv = np.linalg.inv(XtX)

    def resid(V):
        return V - X64 @ (XtX_inv @ (X64.T @ V))

    Gt = resid(G.astype(np.float64))                 # [N, L]
    gtg = np.sum(Gt * Gt, axis=0)                    # [L]
    Gn = (Gt / np.sqrt(gtg)).astype(np.float32)      # [N, L] unit-norm columns

    Yp = _permuted_y(y, key_seed, P).T               # [N, P]
    Yt64 = resid(Yp.astype(np.float64))
    yty = np.sum(Yt64 * Yt64, axis=0)                # [P] float64
    Yt = Yt64.astype(np.float32)

    if mm_dtype_name == "float16":
        npdt = np.float16
        cast = lambda a: a.astype(np.float16)
    else:
        npdt = np.float32
        cast = _round_tf32

    AW = PAD_PERM + CHUNKS[0]
    BW = CHUNKS[1] + CHUNKS[2]
    Ytq = cast(Yt)                                   # [n, P]
    Gq = cast(Gn).reshape(n, N_CORES, L_SHARD).transpose(1, 0, 2)  # [cores, n, 1250]

    pa = np.zeros((N_CORES, PAD_SAMP, AW), dtype=npdt)
    pa[:, :n, 0:CHUNKS[0]] = Gq[:, :, 0:512]
    pa[:, :n, CHUNKS[0]:CHUNKS[0] + P] = Ytq[None]
    pb = np.zeros((N_CORES, PAD_SAMP, BW), dtype=npdt)
    pb[:, :n, 0:L_SHARD - 512] = Gq[:, :, 512:L_SHARD]
    in_maps = [
        {
            "pa": pa[c].reshape(KTILES, 128, AW),
            "pb": pb[c].reshape(KTILES, 128, BW),
        }
        for c in range(N_CORES)
    ]
    return in_maps, yty


def postprocess(results, yty, obs_p, P, n=N_SAMP, k_cov=N_COV):
    """Gather device outputs and finish the tiny fp32 tail like the reference."""
    import jax
    import jax.numpy as jnp
    import jax.scipy.stats as jaxstats

    smax_cores = np.stack([
        np.asarray(r["smax"]).T.reshape(-1)[:P] for r in results
    ])                                               # [cores, P] max|C|
    s = np.max(smax_cores, axis=0).astype(np.float64) ** 2

    dof = float(n - k_cov - 1)
    TS = np.sqrt(dof * s / (yty - s)).astype(np.float32)
    cpu = jax.devices("cpu")[0]
    with jax.default_device(cpu):
        pvals = np.asarray(
            1.0 - jaxstats.chi2.cdf(jnp.square(jnp.asarray(TS)), 1.0),
            dtype=np.float32,
        )
    obs = np.asarray(obs_p, dtype=np.float32)
    adj_p = np.float32((np.sum(pvals < obs) + 1) / (P + 1))
    return adj_p, pvals, TS


def kernel(X, y, G, obs_p, key_seed, max_perm_direct):
    from concourse.bass_utils import run_bass_kernel_spmd

    P = int(np.asarray(max_perm_direct))
    seed = int(np.asarray(key_seed))
    in_maps, yty = prepare_inputs(X, y, G, seed, P)
    nc = _get_nc()
    res = run_bass_kernel_spmd(nc, in_maps, list(range(N_CORES)))
    return postprocess(res.results, yty, obs_p, P)
